# revision 33
# baseline (speedup 1.0000x reference)
"""AdjacencyAwareMultiHeadAttention on 8 trn2 NeuronCores.

Sharding: data-parallel over batch. Core b handles graph b entirely
(all 8 heads). Host does input repacking (transposes / dtype prep),
final normalization by the softmax denominator, and the 4 scalar
diagnostic means.

Device math per core (graph), S^T layout ([m=key on partitions, n=query free]):
  QT/KT = WT.T @ hT (+bias)            [256, 1024] f32 (two 128-row blocks)
  V     = hT.T @ WvT (+bias)           [1024, 264] with a ones column per head
  per (head, n-chunk, m-tile):
    S^T   = KT_h_slice.T @ QT_h_slice          (PSUM, f32)
    P     = exp(S * 1/sqrt(D) + keypad_bias)   (ScalarE, -> bf16 SBUF)
    P'    = P * alpha^A                        (VectorE, bf16)
    T0,T1 = P' * (A==0), P' * (A==1)           (VectorE, bf16)
    O'   += [V_h | 1].T @ P'   (rows 0-31: unnormalized out^T, row 32: colsum P')
    Z    += ones.T @ P         (softmax denominator)
    U0   += ones.T @ T0 ; U1 += ones.T @ T1
Host: O = O'[0:32]/Z, diagnostics from Z/U0/U1/colsumP' and exact mask
counts of A.
"""

import math
import os
import sys

import numpy as np

sys.path.insert(0, "/opt/trn_rl_repo")

import concourse.bass as bass
import concourse.mybir as mybir
from concourse.tile import TileContext
from concourse import bacc, bass_utils

B, N, IND, D, H = 8, 1024, 128, 32, 8
HD = H * D  # 256
NT = N // 128  # 8 m-tiles
NCH = N // 512  # 2 n-chunks
SCALE = 1.0 / math.sqrt(D)
NEG = -60.0  # key-pad bias: exp(-60) == 0 for our logit range

F32 = mybir.dt.float32
BF16 = mybir.dt.bfloat16

_last_exec_time_ns = None


# column offsets inside the packed [128, IN_COLS] input
# WQ/WK are head-replicated per 2-head block: block b's 128 weight
# columns produce QTR rows [h0,h0,h1,h1] x 32 so four S-matmuls can run
# in distinct PE row-groups concurrently.
OFF_HT = 0
OFF_A = 1024
OFF_KB = OFF_A + NT * N          # 9216
OFF_WQ = OFF_KB + NT             # 9224  [128, 512] replicated (4 blk x 128)
OFF_WK = OFF_WQ + 512            # 9736
OFF_WV = OFF_WK + 512            # 10248 [128, 256]
OFF_BQ = OFF_WV + HD             # 10504 [128, 4] replicated
OFF_BK = OFF_BQ + 4              # 10508
OFF_BV = OFF_BK + 4              # 10512
IN_COLS = OFF_BV + HD            # 10768


def _build(ln_alpha: float) -> bass.Bass:
    nc = bacc.Bacc()

    in_d = nc.dram_tensor("IN", [128, IN_COLS], F32, kind="ExternalInput")

    O_d = nc.dram_tensor("O", [H, 33, N], F32, kind="ExternalOutput")
    Zu_d = nc.dram_tensor("Zu", [H, 3, N], F32, kind="ExternalOutput")

    with TileContext(nc) as tc:
        with (
            tc.tile_pool(name="const", bufs=1) as cpool,
            tc.tile_pool(name="stage", bufs=3) as spool,
            tc.tile_pool(name="ppool", bufs=3) as ppool,
            tc.tile_pool(name="psum_big", bufs=3, space="PSUM") as ps_big,
            tc.tile_pool(name="psum_o", bufs=2, space="PSUM") as ps_o,
            tc.tile_pool(name="psum_row", bufs=3, space="PSUM") as ps_row,
        ):
            # ---- load all inputs in one DMA ----
            IN = cpool.tile([128, IN_COLS], F32, tag="IN")
            nc.sync.dma_start(IN[:], in_d[:])
            hT = IN[:, OFF_HT:OFF_HT + N]
            Af = IN[:, OFF_A:OFF_A + NT * N]
            kb = IN[:, OFF_KB:OFF_KB + NT]
            wqT = IN[:, OFF_WQ:OFF_WQ + 512]
            wkT = IN[:, OFF_WK:OFF_WK + 512]
            wvT = IN[:, OFF_WV:OFF_WV + HD]
            bqT = IN[:, OFF_BQ:OFF_BQ + 4]
            bkT = IN[:, OFF_BK:OFF_BK + 4]
            bvR = IN[:, OFF_BV:OFF_BV + HD]

            ones = cpool.tile([128, 1], BF16, tag="ones")
            nc.vector.memset(ones[:], 1.0)

            # ---- A-derived tiles: W = alpha^A, M0 = (A==0), M1 = (A==1) ----
            Wsb = cpool.tile([128, NT * N], BF16, tag="Wsb")
            M0 = cpool.tile([128, NT * N], BF16, tag="M0")
            M1 = cpool.tile([128, NT * N], BF16, tag="M1")
            for mi in range(NT):
                for ch in range(NCH):
                    sl = bass.ds(mi * N + ch * 512, 512)
                    nc.scalar.activation(
                        Wsb[:, sl], Af[:, sl],
                        mybir.ActivationFunctionType.Exp, scale=ln_alpha,
                    )
                    nc.vector.tensor_scalar(
                        out=M0[:, sl], in0=Af[:, sl], scalar1=0.0, scalar2=None,
                        op0=mybir.AluOpType.is_equal,
                    )
                    nc.vector.tensor_scalar(
                        out=M1[:, sl], in0=Af[:, sl], scalar1=1.0, scalar2=None,
                        op0=mybir.AluOpType.is_equal,
                    )

            # ---- projections (head-replicated layout) ----
            # QTR[blk] rows: [h0,h0,h1,h1] x 32 where h0 = 2*blk
            QTR = [cpool.tile([128, N], F32, tag=f"QTR{b}", name=f"QTR{b}")
                   for b in range(4)]
            KTR = [cpool.tile([128, N], F32, tag=f"KTR{b}", name=f"KTR{b}")
                   for b in range(4)]
            for b in range(4):
                for ch in range(NCH):
                    nsl = bass.ds(ch * 512, 512)
                    csl = bass.ds(b * 128, 128)
                    pq = ps_big.tile([128, 512], F32, tag="bigp")
                    nc.tensor.matmul(pq[:], wqT[:, csl], hT[:, nsl],
                                     start=True, stop=True)
                    nc.vector.tensor_scalar_add(QTR[b][:, nsl], pq[:],
                                                bqT[:, b:b + 1])
                    pk = ps_big.tile([128, 512], F32, tag="bigp")
                    nc.tensor.matmul(pk[:], wkT[:, csl], hT[:, nsl],
                                     start=True, stop=True)
                    nc.vector.tensor_scalar_add(KTR[b][:, nsl], pk[:],
                                                bkT[:, b:b + 1])

            # V with ones column: Vp1[p, mi*264 + h*33 + d], d=32 -> 1.0
            Vp1 = cpool.tile([128, NT * 264], BF16, tag="Vp1")
            nc.vector.memset(Vp1[:], 1.0)
            for mi in range(NT):
                pv = ps_big.tile([128, HD], F32, tag="bigp")
                nc.tensor.matmul(pv[:], hT[:, bass.ds(mi * 128, 128)], wvT[:],
                                 start=True, stop=True)
                vtmp = ps_big.tile([128, HD], F32, tag="bigp")
                nc.vector.tensor_add(vtmp[:], pv[:], bvR[:])
                dst = Vp1[:, bass.ds(mi * 264, 264)].rearrange(
                    "p (h x) -> p h x", x=33)[:, :, 0:D]
                nc.vector.tensor_copy(
                    dst, vtmp[:].rearrange("p (h d) -> p h d", d=D))

            # ---- main attention loops: 2-head blocks ----
            # Op: head0 [V|1]-out at partitions 0..32, head1 at 64..96
            # rowsA: Z(h0)@0, Z(h1)@32, U0(h0)@64, U0(h1)@96
            # rowsB: U1(h0)@0, U1(h1)@32
            for blk in range(H // 2):
                h0 = 2 * blk
                c = h0 // 4
                for ch in range(NCH):
                    nsl = bass.ds(ch * 512, 512)
                    Op = ps_o.tile([97, 512], F32, tag="Op")
                    rowsA = ps_row.tile([97, 512], F32, tag="rows", name="rA")
                    rowsB = ps_row.tile([33, 512], F32, tag="rows", name="rB")
                    for mi in range(NT):
                        first, last = mi == 0, mi == NT - 1
                        asl = bass.ds(mi * N + ch * 512, 512)
                        Ps, Pps = [], []
                        for side in range(2):
                            h = h0 + side
                            rg = 2 * side
                            Sp = ps_big.tile([128, 512], F32, tag="bigp")
                            nc.tensor.matmul(
                                Sp[:],
                                KTR[blk][bass.ds(rg * 32, 32),
                                         bass.ds(mi * 128, 128)],
                                QTR[blk][bass.ds(rg * 32, 32), nsl],
                                start=True, stop=True,
                                tile_position=(rg * 32, 0),
                            )
                            P = ppool.tile([128, 512], BF16, tag=f"P{side}")
                            nc.scalar.activation(
                                P[:], Sp[:], mybir.ActivationFunctionType.Exp,
                                bias=kb[:, mi:mi + 1], scale=SCALE,
                            )
                            Pp = ppool.tile([128, 512], BF16, tag=f"Pp{side}")
                            nc.vector.tensor_mul(Pp[:], P[:], Wsb[:, asl])
                            Ps.append(P)
                            Pps.append(Pp)
                        for side in range(2):
                            h = h0 + side
                            P, Pp = Ps[side], Pps[side]
                            T0 = ppool.tile([128, 512], BF16, tag=f"T0{side}")
                            nc.any.tensor_tensor(
                                out=T0[:], in0=Pp[:], in1=M0[:, asl],
                                op=mybir.AluOpType.mult)
                            T1 = ppool.tile([128, 512], BF16, tag=f"T1{side}")
                            nc.any.tensor_tensor(
                                out=T1[:], in0=Pp[:], in1=M1[:, asl],
                                op=mybir.AluOpType.mult)
                            o = 64 * side
                            nc.tensor.matmul(
                                Op[o:o + 33, :],
                                Vp1[:, bass.ds(mi * 264 + h * 33, 33)], Pp[:],
                                start=first, stop=last,
                                tile_position=(0, o))
                            q = 32 * side
                            nc.tensor.matmul(
                                rowsA[q:q + 1, :], ones[:], P[:],
                                start=first, stop=last, tile_position=(0, q))
                            nc.tensor.matmul(
                                rowsA[64 + q:65 + q, :], ones[:], T0[:],
                                start=first, stop=last,
                                tile_position=(0, 64 + q))
                            nc.tensor.matmul(
                                rowsB[q:q + 1, :], ones[:], T1[:],
                                start=first, stop=last, tile_position=(0, q))
                    Ostg = spool.tile([97, 512], F32, tag="Ostg")
                    nc.any.tensor_copy(Ostg[:], Op[:])
                    rstgA = spool.tile([97, 512], F32, tag="rstgA")
                    nc.any.tensor_copy(rstgA[:], rowsA[:])
                    rstgB = spool.tile([33, 512], F32, tag="rstgB")
                    nc.any.tensor_copy(rstgB[:], rowsB[:])
                    for side in range(2):
                        h = h0 + side
                        o = 64 * side
                        q = 32 * side
                        nc.sync.dma_start(O_d[h, 0:33, nsl],
                                          Ostg[o:o + 33, :])
                        nc.sync.dma_start(Zu_d[h, 0:1, nsl],
                                          rstgA[q:q + 1, :])
                        nc.sync.dma_start(Zu_d[h, 1:2, nsl],
                                          rstgA[64 + q:65 + q, :])
                        nc.sync.dma_start(Zu_d[h, 2:3, nsl],
                                          rstgB[q:q + 1, :])
    return nc


def kernel(h, A, lengths, alpha, Wq, bq, Wk, bk, Wv, bv):
    global _last_exec_time_ns
    h = np.asarray(h, np.float32)
    A = np.asarray(A)
    lengths = np.asarray(lengths)
    alpha_v = float(np.asarray(alpha).reshape(-1)[0])
    ln_alpha = math.log(alpha_v)

    wqT = np.asarray(Wq, np.float32).T  # [128, 256]
    wkT = np.asarray(Wk, np.float32).T
    wvT = np.ascontiguousarray(np.asarray(Wv, np.float32).T)
    bvR = np.tile(np.asarray(bv, np.float32)[None, :], (128, 1))
    # head-replicated weight columns: block b -> [h0,h0,h1,h1] x 32
    ridx = np.concatenate([
        np.concatenate([np.arange(h * D, (h + 1) * D)
                        for h in (2 * b, 2 * b, 2 * b + 1, 2 * b + 1)])
        for b in range(4)])  # [512]
    wqTR = np.ascontiguousarray(wqT[:, ridx].reshape(128, 4, 128)
                                .reshape(128, 512))
    wkTR = np.ascontiguousarray(wkT[:, ridx].reshape(128, 4, 128)
                                .reshape(128, 512))
    bq_a = np.asarray(bq, np.float32)
    bk_a = np.asarray(bk, np.float32)
    bqTR = np.ascontiguousarray(bq_a[ridx].reshape(4, 128).T)  # [128, 4]
    bkTR = np.ascontiguousarray(bk_a[ridx].reshape(4, 128).T)

    nc = _build(ln_alpha)
    nc.finalize()

    in_maps = []
    for b in range(B):
        IN = np.empty((128, IN_COLS), np.float32)
        IN[:, OFF_HT:OFF_HT + N] = h[b].T
        # Af[p, mi*N + n] = A[b, n, mi*128+p]  (att is reweighted by
        # alpha^A[query, key]; our tiles are [key, query])
        IN[:, OFF_A:OFF_A + NT * N] = (
            np.ascontiguousarray(A[b].T).astype(np.float32)
            .reshape(NT, 128, N).transpose(1, 0, 2).reshape(128, NT * N))
        kbv = np.where(np.arange(N) < int(lengths[b]), 0.0, NEG)
        IN[:, OFF_KB:OFF_KB + NT] = kbv.reshape(NT, 128).T
        IN[:, OFF_WQ:OFF_WQ + 512] = wqTR
        IN[:, OFF_WK:OFF_WK + 512] = wkTR
        IN[:, OFF_WV:OFF_WV + HD] = wvT
        IN[:, OFF_BQ:OFF_BQ + 4] = bqTR
        IN[:, OFF_BK:OFF_BK + 4] = bkTR
        IN[:, OFF_BV:OFF_BV + HD] = bvR
        in_maps.append({"IN": IN})

    trace = bool(os.environ.get("KERNEL_TRACE"))
    try:
        res = bass_utils.run_bass_kernel_spmd(
            nc, in_maps, core_ids=list(range(B)), trace=trace)
    except ModuleNotFoundError:
        res = bass_utils.run_bass_kernel_spmd(
            nc, in_maps, core_ids=list(range(B)), trace=False)
    _last_exec_time_ns = getattr(res, "exec_time_ns", None)
    if _last_exec_time_ns is not None:
        print(f"HW exec time: {_last_exec_time_ns} ns")
    outs = res.results
    global _last_outs
    _last_outs = outs

    # ---- host-side gather / normalize / diagnostics ----
    h_heads = np.zeros((B, H, N, D), np.float32)
    U0 = U1 = F = 0.0
    for b in range(B):
        O = np.asarray(outs[b]["O"], np.float32)      # [H, 33, N]
        Zu = np.asarray(outs[b]["Zu"], np.float32)    # [H, 3, N]
        ln = int(lengths[b])
        Z = Zu[:, 0, :]                               # [H, N]
        rz = np.zeros_like(Z)
        rz[:, :ln] = 1.0 / Z[:, :ln]
        h_heads[b] = (O[:, 0:D, :] * rz[:, None, :]).transpose(0, 2, 1)
        F += float((O[:, D, :] * rz).sum())
        U0 += float((Zu[:, 1, :] * rz).sum())
        U1 += float((Zu[:, 2, :] * rz).sum())

    cnt1 = float(H) * float(np.count_nonzero(A == 1))
    cnt2 = float(H) * float(np.count_nonzero(A > 1))
    S_tot = float(H) * float(np.sum(lengths))
    S1 = U1 / alpha_v
    pre_d1 = S1 / cnt1
    pre_d2 = (S_tot - U0 - S1) / cnt2
    post_d1 = U1 / cnt1
    post_d2 = (F - U0 - U1) / cnt2
    return (h_heads, np.float32(pre_d1), np.float32(pre_d2),
            np.float32(post_d1), np.float32(post_d2))
